# revision 36
# baseline (speedup 1.0000x reference)
"""Causal multi-head attention block (B=8, T=2048, C=768, H=8) on 8 trn2 cores.

Sharding: data-parallel over batch — one batch element per NeuronCore, weights
replicated, no collectives.

Per-core algorithm:
  Phase A: PE-transpose x_b -> x^T [c, t] in SBUF (bf16); w_attn -> transposed
           SBUF-resident waT (bf16); w_proj -> transposed DRAM scratch (f32r).
  Phase B: per superblock of 4 heads: V = x @ w_v^T + b_v (natural [t, d]
           layout, with an appended ones column per head for the softmax
           denominator); per head: Q^T/K^T = (w x^T) + b (head-aligned [d, t]
           layout, Q pre-scaled by 1/sqrt(hs) via prescaled weights); causal
           attention in S^T layout:
             S^T[j, i] = K^T.T-free matmul; P = exp(S^T) on ACT;
             diagonal-block mask multiply on DVE;
             O^T[d, i] (+ denominator row l) accumulated in PSUM over j-tiles
             via lhsT=[V|1]; normalize by 1/l via the single-pass DVE
             reciprocal_approx_fast (~5x faster than the exact iterative
             divide, keeps the ACT queue free for exp), spill O^T to DRAM
             per i-half so phase C can start early.
  Phase C: out = oT.T @ w_proj^T + b_proj with K=128 contraction tiles,
           written [t, c]; shares the phase-B scope so its matmuls overlap
           the tail of the attention.
"""

import math
import os
import sys
from contextlib import ExitStack

for _p in ("/opt/trn_rl_repo", "/root/.axon_site/_ro/trn_rl_repo"):
    if os.path.isdir(_p) and _p not in sys.path:
        sys.path.append(_p)

import numpy as np
import ml_dtypes

import concourse.bass as bass  # noqa: F401  (import keeps bass registered)
from concourse import bacc
import concourse.mybir as mybir
import concourse.tile as tile
from concourse.bass_utils import run_bass_kernel_spmd

F32 = mybir.dt.float32
F32R = mybir.dt.float32r
BF16 = mybir.dt.bfloat16
EXP = mybir.ActivationFunctionType.Exp
LN = mybir.ActivationFunctionType.Ln
ADD = mybir.AluOpType.add
MULT = mybir.AluOpType.mult

B, T, C, H, HS = 8, 2048, 768, 8, 96
KT = C // 128        # 6 contraction tiles of 128
TT = T // 128        # 16 t-tiles of 128
NCORES = 8


def _chunks(lo, hi, align=512):
    """Split [lo, hi) at multiples of `align`."""
    out = []
    a = lo
    while a < hi:
        b = min(hi, (a // align + 1) * align)
        out.append((a, b))
        a = b
    return out


def build_nc():
    nc = bacc.Bacc()
    x_b = nc.dram_tensor("x_b", [T, C], BF16, kind="ExternalInput")
    wat = nc.dram_tensor("wat", [3 * C, C], BF16, kind="ExternalInput")
    wp = nc.dram_tensor("wp", [C, C], F32R, kind="ExternalInput")
    ident = nc.dram_tensor("ident", [128, 128], BF16, kind="ExternalInput")
    identf = nc.dram_tensor("identf", [128, 128], F32R, kind="ExternalInput")
    mk = nc.dram_tensor("mk", [128, 128], F32R, kind="ExternalInput")
    bsel = nc.dram_tensor("bsel", [128, HS], F32R, kind="ExternalInput")
    bqk = nc.dram_tensor("bqk", [HS, 16], F32, kind="ExternalInput")
    bv = nc.dram_tensor("bv", [128, C], F32, kind="ExternalInput")
    bo = nc.dram_tensor("bo", [128, C], F32, kind="ExternalInput")
    out = nc.dram_tensor("out", [T, C], F32, kind="ExternalOutput")

    with tile.TileContext(nc) as tc, ExitStack() as ctx:

        consts = ctx.enter_context(tc.tile_pool(name="consts", bufs=1))
        id_sb = consts.tile([128, 128], BF16, tag="id")
        idf_sb = consts.tile([128, 128], F32R, tag="idf")
        mk_sb = consts.tile([128, 128], F32R, tag="mk")
        bs_sb = consts.tile([128, HS], F32R, tag="bs")
        bqk_sb = consts.tile([HS, 16], F32, tag="bqk")
        bv_sb = consts.tile([128, C], F32, tag="bv")
        bo_sb = consts.tile([128, C], F32, tag="bo")
        one_f32 = consts.tile([128, 1], F32, tag="one")
        nc.vector.memset(one_f32[:], 1.0)
        # ident first: the very first transposes need it; other consts follow
        # the big phase-A loads so they don't delay the critical path
        nc.sync.dma_start(id_sb[:], ident[:, :])

        xTp = ctx.enter_context(tc.tile_pool(name="xT", bufs=1))
        xT = xTp.tile([128, KT, T], BF16, tag="xT")
        waTp = ctx.enter_context(tc.tile_pool(name="waT", bufs=1))
        waT = waTp.tile([128, KT, 3 * C], BF16, tag="waT")   # w_attn^T resident
        # w_proj^T and the attention output O^T stay SBUF-resident end to end:
        # the transpose evictions write wpT_sb directly, and per-head O^T
        # pieces are partition-shifted into the K=128-packed oS stripes that
        # phase C consumes — no DRAM round trip.
        wpTp = ctx.enter_context(tc.tile_pool(name="wpTsb", bufs=1))
        wpT_sb = wpTp.tile([128, KT, C], BF16, tag="wpTsb")
        oSp = ctx.enter_context(tc.tile_pool(name="oS", bufs=1))
        oS = oSp.tile([128, KT, T], BF16, tag="oS")

        # ---------------- Phase A: transposes ----------------
        # Sources are DMAd in stripes sized so the PE transposes start early;
        # transposed-row accumulation happens in PSUM, one eviction copy per
        # 128-row stripe of the transposed tensor.
        with tc.tile_pool(name="pa_in", bufs=3) as pin, \
             tc.tile_pool(name="pa_ps", bufs=4, space="PSUM") as pps, \
             tc.tile_pool(name="pa_pw", bufs=2, space="PSUM") as ppw:
            # w_attn first (its transpose feeds the projections): the V-head
            # stripes, then interleaved q/k stripes so early heads unblock
            # first; x after; w_proj last (only needed in phase C).
            wat_r = wat.rearrange("(a p) c -> p a c", p=128)
            x_r = x_b.rearrange("(a p) c -> p a c", p=128)

            def tr_group(src, sls, kc, dst, psp, idt, dt):
                """Transpose len(sls) consecutive 128-blocks of `src` stripes
                into one wide PSUM tile, then evict with a single copy."""
                n = len(sls)
                psb = psp.tile([128, n * 128], dt, tag="tps")
                bank = 1024 if dt == BF16 else 512
                for i, sl in enumerate(sls):
                    first = (i == 0) or \
                        (i * 128) // bank != ((i * 128 - 1) // bank)
                    last = (i == n - 1) or \
                        ((i + 1) * 128 - 1) // bank != (((i + 2) * 128 - 1) // bank)
                    nc.tensor.matmul(psb[:, i * 128:(i + 1) * 128],
                                     src[:, sl, kc * 128:(kc + 1) * 128],
                                     idt[:], is_transpose=True,
                                     start=first, stop=last)
                nc.any.tensor_copy(dst, psb[:])

            def w_stripes(tile_, rt_groups):
                for kc in range(KT):
                    for (sl0, rt0, n) in rt_groups:
                        tr_group(tile_, range(sl0, sl0 + n), kc,
                                 waT[:, kc, rt0 * 128:(rt0 + n) * 128],
                                 pps, id_sb, BF16)

            vch = pin.tile([128, 6, C], BF16, tag="ain", name="vch")
            nc.sync.dma_start(vch[:, 0:3, :], wat_r[:, 12:15, :])
            nc.sync.dma_start(vch[:, 3:6, :], wat_r[:, 15:18, :])
            nc.sync.dma_start(mk_sb[:], mk[:, :])
            nc.sync.dma_start(bs_sb[:], bsel[:, :])
            nc.sync.dma_start(bqk_sb[:], bqk[:, :])
            nc.sync.dma_start(bv_sb[:], bv[:, :])
            nc.sync.dma_start(bo_sb[:], bo[:, :])
            nc.sync.dma_start(idf_sb[:], identf[:, :])
            w_stripes(vch, [(0, 12, 3), (3, 15, 3)])

            def x_chunk(ch):
                xch = pin.tile([128, 6, C], BF16, tag="ain", name=f"xch{ch}")
                nch = 6 if ch < 2 else 4
                nh = nch // 2
                nc.scalar.dma_start(xch[:, 0:nh, :],
                                    x_r[:, ch * 6:ch * 6 + nh, :])
                nc.scalar.dma_start(xch[:, nh:nch, :],
                                    x_r[:, ch * 6 + nh:ch * 6 + nch, :])
                for kc in range(KT):
                    for (s0, s1) in ((0, nh), (nh, nch)):
                        tr_group(xch, range(s0, s1), kc,
                                 xT[:, kc, (ch * 6 + s0) * 128:
                                    (ch * 6 + s1) * 128],
                                 pps, id_sb, BF16)

            x_chunk(0)
            # early heads' q/k stripes land between the x chunks so head 0's
            # projection can start as soon as the first x chunk is transposed
            qk0 = pin.tile([128, 6, C], BF16, tag="ain", name="qk0")
            nc.sync.dma_start(qk0[:, 0:3, :], wat_r[:, 0:3, :])
            nc.sync.dma_start(qk0[:, 3:6, :], wat_r[:, 6:9, :])
            w_stripes(qk0, [(0, 0, 3), (3, 6, 3)])
            x_chunk(1)
            qk1 = pin.tile([128, 6, C], BF16, tag="ain", name="qk1")
            nc.sync.dma_start(qk1[:, 0:3, :], wat_r[:, 3:6, :])
            nc.sync.dma_start(qk1[:, 3:6, :], wat_r[:, 9:12, :])
            w_stripes(qk1, [(0, 3, 3), (3, 9, 3)])
            x_chunk(2)
            wpin = pin.tile([128, 6, C], F32R, tag="win", name="wpin", bufs=1)
            nc.gpsimd.dma_start(wpin[:], wp.rearrange("(a p) c -> p a c", p=128))
            for kc in range(KT):
                tr_group(wpin, range(C // 128), kc, wpT_sb[:, kc, :],
                         ppw, idf_sb, F32R)

        # ------- Phase B: projections + attention;  Phase C: out proj -------
        # One scope so phase C's matmuls overlap the tail of the attention.
        with tc.tile_pool(name="vsb", bufs=2) as vsbp, \
             tc.tile_pool(name="qk", bufs=4) as qkp, \
             tc.tile_pool(name="pt", bufs=2) as ptp, \
             tc.tile_pool(name="ep", bufs=2) as epp, \
             tc.tile_pool(name="pcp", bufs=2) as pcp, \
             tc.tile_pool(name="bps", bufs=2, space="PSUM") as bps, \
             tc.tile_pool(name="pj", bufs=2, space="PSUM") as pjps, \
             tc.tile_pool(name="ops", bufs=1, space="PSUM") as opsp:
            for pr in range(4):
                # V projection for this pair of heads, natural [t, d] layout
                # with an appended ones column per head (softmax denominator).
                # The matmul N is padded to 256 (f32r needs free >= 256 for
                # full rate); the pad may read into neighbouring v columns.
                start_off = min(2 * HS * pr, C - 256)
                off = 2 * HS * pr - start_off
                V = vsbp.tile([128, TT, 2, HS + 1], BF16, tag="V")
                nc.vector.tensor_copy(
                    V.rearrange("p a b c -> p (a b c)"),
                    one_f32[:].to_broadcast([128, TT * 2 * (HS + 1)]))
                for tt in range(TT):
                    vps = pjps.tile([128, 512], F32, tag="pj")
                    for kc in range(KT):
                        nc.tensor.matmul(vps[:, 0:256],
                                         xT[:, kc, tt * 128:(tt + 1) * 128],
                                         waT[:, kc, 2 * C + start_off:
                                             2 * C + start_off + 256],
                                         start=(kc == 0), stop=(kc == KT - 1))
                    nc.vector.tensor_tensor(
                        V[:, tt, :, 0:HS],
                        vps[:, off:off + 2 * HS]
                            .rearrange("p (h d) -> p h d", d=HS),
                        bv_sb[:, 2 * HS * pr:2 * HS * (pr + 1)]
                            .rearrange("p (h d) -> p h d", d=HS),
                        ADD)

                for hh in range(2):
                    h = 2 * pr + hh
                    # Q^T/K^T projection for head h ([d, t] layout); per-head
                    # granularity lets the next head's projection overlap the
                    # current head's (ACT-bound) attention inner loop.
                    qkh = [qkp.tile([128, T], F32R, tag="qk", name=f"qk{i}")
                           for i in range(2)]
                    for tc4 in range(4):
                        for mc in range(2):          # 0 = q, 1 = k
                            wc = h * HS + (0 if mc == 0 else C)
                            pj = pjps.tile([128, 512], F32, tag="pj")
                            for kc in range(KT):
                                nc.tensor.matmul(
                                    pj[0:HS, 0:512],
                                    waT[:, kc, wc:wc + HS],
                                    xT[:, kc, tc4 * 512:(tc4 + 1) * 512],
                                    start=(kc == 0), stop=(kc == KT - 1))
                            m_col = h + (0 if mc == 0 else 8)
                            nc.vector.tensor_tensor(
                                qkh[mc][0:HS, tc4 * 512:(tc4 + 1) * 512],
                                pj[0:HS, 0:512],
                                bqk_sb[:, m_col:m_col + 1].to_broadcast([HS, 512]),
                                ADD)

                    qT, kT = qkh[0], qkh[1]
                    Oe = epp.tile([HS, T], BF16, tag="Oe", bufs=1)
                    for ihalf in range(2):
                        ibase = 1024 * ihalf
                        iend = ibase + 1024
                        njt = 8 * (ihalf + 1)
                        O_ps = opsp.tile([128, 1024], F32, tag="O")
                        for jt in range(njt):
                            j0 = 128 * jt
                            i0 = max(j0, ibase)
                            ilen = iend - i0
                            S = bps.tile([128, 1024], F32, tag="ps")
                            for (ra, rb) in _chunks(0, ilen):
                                nc.tensor.matmul(S[:, ra:rb],
                                                 kT[0:HS, j0:j0 + 128],
                                                 qT[0:HS, i0 + ra:i0 + rb],
                                                 start=True, stop=True)
                            P = ptp.tile([128, 1024], BF16, tag="P")
                            nc.scalar.activation(P[:, 0:ilen], S[:, 0:ilen],
                                                 EXP)
                            if j0 >= ibase:
                                nc.gpsimd.tensor_tensor(P[:, 0:128],
                                                        P[:, 0:128],
                                                        mk_sb[:], MULT)
                            for (a, b) in _chunks(i0, iend):
                                ci = a // 512
                                last_jt = min(4 * ci + 3, njt - 1)
                                nc.tensor.matmul(
                                    O_ps[0:HS + 1, a - ibase:b - ibase],
                                    V[:, jt, hh, :],
                                    P[:, a - i0:b - i0],
                                    start=(jt == 0), stop=(jt == last_jt))
                        # epilogue: normalize by the denominator row l
                        # (broadcast by selector matmul, then a single-pass
                        # approximate reciprocal on DVE — ~5x faster than the
                        # exact iterative divide, ~51 ULP).
                        lt = epp.tile([HS + 1, 1024], F32R, tag="lt",
                                      bufs=1)
                        nc.scalar.copy(lt[:], O_ps[0:HS + 1, :])
                        Lp = bps.tile([128, 1024], F32, tag="ps")
                        for (ra, rb) in ((0, 512), (512, 1024)):
                            nc.tensor.matmul(Lp[0:HS, ra:rb],
                                             bs_sb[0:HS + 1, :],
                                             lt[:, ra:rb],
                                             start=True, stop=True)
                        R = epp.tile([HS, 1024], F32, tag="R", bufs=1)
                        nc.vector.reciprocal_approx_fast(R[:], Lp[0:HS, :])
                        nc.gpsimd.tensor_tensor(Oe[:, ibase:iend],
                                                lt[0:HS, :], R[:], MULT)
                        # shift this i-half into the K=128-packed stripes
                        # (SBUF->SBUF DMA does the partition remap)
                        r0 = h * HS
                        k0, off = r0 // 128, r0 % 128
                        n0 = min(HS, 128 - off)
                        nc.gpsimd.dma_start(
                            oS[off:off + n0, k0, ibase:iend],
                            Oe[0:n0, ibase:iend])
                        if n0 < HS:
                            nc.gpsimd.dma_start(
                                oS[0:HS - n0, k0 + 1, ibase:iend],
                                Oe[n0:HS, ibase:iend])

            # ---------------- Phase C: output projection ----------------
            # O^T streamed back from DRAM per 2-t-tile block (pipelined),
            # K=128 contraction tiles (128-row stripes of the concatenated
            # head dim), PSUM shared with the projection pool.
            out_r = out.rearrange("(g a p) c -> p g a c", a=2, p=128)
            # tg 0..3 (i < 1024) are fully unblocked once every head's first
            # i-half is done; run them whole.
            for tg in range(TT // 4):
                for ta in range(2):
                    t0 = tg * 256 + ta * 128
                    o_sb = pcp.tile([128, C], F32, tag="osb")
                    for (a, b) in ((0, 512), (512, C)):
                        cps = pjps.tile([128, 512], F32, tag="pj")
                        for kc in range(KT):
                            nc.tensor.matmul(cps[:, 0:b - a],
                                             oS[:, kc, t0:t0 + 128],
                                             wpT_sb[:, kc, a:b],
                                             start=(kc == 0), stop=(kc == KT - 1))
                        nc.vector.tensor_tensor(o_sb[:, a:b],
                                                cps[:, 0:b - a],
                                                bo_sb[:, a:b], ADD)
                    nc.gpsimd.dma_start(out_r[:, tg, ta], o_sb[:])
            # tg 4..7 (i >= 1024) would otherwise serialize behind the last
            # head's final i-half: contract kc 0..4 (+bias) into SBUF partials
            # while that head still runs, leaving only the kc=5 pass + add on
            # the tail.  All stage-1 allocations precede stage-2 so the psum
            # pool rotation never parks an early tile behind a late one.
            parts = {}
            for tg in range(TT // 4, TT // 2):
                for ta in range(2):
                    t0 = tg * 256 + ta * 128
                    part = pcp.tile([128, C], F32, tag="part", bufs=8)
                    parts[(tg, ta)] = part
                    for (a, b) in ((0, 512), (512, C)):
                        cps = pjps.tile([128, 512], F32, tag="pj")
                        for kc in range(KT - 1):
                            nc.tensor.matmul(cps[:, 0:b - a],
                                             oS[:, kc, t0:t0 + 128],
                                             wpT_sb[:, kc, a:b],
                                             start=(kc == 0), stop=(kc == KT - 2))
                        nc.vector.tensor_tensor(part[:, a:b],
                                                cps[:, 0:b - a],
                                                bo_sb[:, a:b], ADD)
            for tg in range(TT // 4, TT // 2):
                for ta in range(2):
                    t0 = tg * 256 + ta * 128
                    part = parts[(tg, ta)]
                    o_sb = pcp.tile([128, C], F32, tag="osb")
                    for (a, b) in ((0, 512), (512, C)):
                        cps = pjps.tile([128, 512], F32, tag="pj")
                        nc.tensor.matmul(cps[:, 0:b - a],
                                         oS[:, KT - 1, t0:t0 + 128],
                                         wpT_sb[:, KT - 1, a:b],
                                         start=True, stop=True)
                        nc.vector.tensor_tensor(o_sb[:, a:b],
                                                cps[:, 0:b - a],
                                                part[:, a:b], ADD)
                    nc.gpsimd.dma_start(out_r[:, tg, ta], o_sb[:])

    nc.finalize()
    return nc


_NC_CACHE = {}


def _get_nc():
    if "nc" not in _NC_CACHE:
        _NC_CACHE["nc"] = build_nc()
    return _NC_CACHE["nc"]


def _make_consts(b_attn, b_proj):
    s = 1.0 / math.sqrt(HS)
    bqk = np.empty((HS, 16), dtype=np.float32)
    for m in range(8):
        bqk[:, m] = b_attn[m * HS:(m + 1) * HS] * s
    for m in range(8):
        bqk[:, 8 + m] = b_attn[C + m * HS:C + (m + 1) * HS]
    bv = np.ascontiguousarray(
        np.broadcast_to(b_attn[2 * C:3 * C], (128, C)).astype(np.float32))
    bo = np.ascontiguousarray(
        np.broadcast_to(b_proj, (128, C)).astype(np.float32))
    ident = np.eye(128, dtype=np.float32)
    mk = np.triu(np.ones((128, 128), dtype=np.float32))
    bsel = np.zeros((128, HS), dtype=np.float32)
    bsel[HS, :] = 1.0
    return bqk, bv, bo, ident, mk, bsel


def kernel(x, w_attn, b_attn, w_proj, b_proj, _want_results=False, **run_kwargs):
    x = np.asarray(x, dtype=np.float32)
    w_attn = np.asarray(w_attn, dtype=np.float32)
    b_attn = np.asarray(b_attn, dtype=np.float32)
    w_proj = np.asarray(w_proj, dtype=np.float32)
    b_proj = np.asarray(b_proj, dtype=np.float32)

    s = 1.0 / math.sqrt(HS)
    wat = w_attn.copy()
    wat[0:C, :] *= s            # fold the 1/sqrt(hs) logit scale into Q
    wat_bf = wat.astype(ml_dtypes.bfloat16)
    x_bf = x.astype(ml_dtypes.bfloat16)
    bqk, bv, bo, ident, mk, bsel = _make_consts(b_attn, b_proj)

    nc = _get_nc()
    common = dict(wat=wat_bf, wp=w_proj,
                  ident=ident.astype(ml_dtypes.bfloat16), identf=ident,
                  mk=mk, bsel=bsel, bqk=bqk, bv=bv, bo=bo)
    in_maps = [dict(x_b=np.ascontiguousarray(x_bf[c]), **common)
               for c in range(NCORES)]
    res = run_bass_kernel_spmd(nc, in_maps, core_ids=list(range(NCORES)),
                               **run_kwargs)
    out = np.stack([res.results[c]["out"] for c in range(NCORES)], axis=0)
    if _want_results:
        return out, res
    return out


if __name__ == "__main__":
    rng = np.random.default_rng(0)
    x = rng.standard_normal((B, T, C), dtype=np.float32)
    w_attn = rng.standard_normal((3 * C, C), dtype=np.float32) / math.sqrt(C)
    b_attn = rng.standard_normal(3 * C).astype(np.float32) * 0.02
    w_proj = rng.standard_normal((C, C), dtype=np.float32) / math.sqrt(C)
    b_proj = rng.standard_normal(C).astype(np.float32) * 0.02
    o = kernel(x, w_attn, b_attn, w_proj, b_proj)
    print("out", o.shape, o.dtype, float(np.abs(o).mean()))


# revision 37
# speedup vs baseline: 1.1727x; 1.1727x over previous
"""Causal multi-head attention block (B=8, T=2048, C=768, H=8) on 8 trn2 cores.

Sharding: data-parallel over batch — one batch element per NeuronCore, weights
replicated, no collectives.

Per-core algorithm:
  Phase A: PE-transpose x_b -> x^T [c, t] in SBUF (bf16); w_attn -> transposed
           SBUF-resident waT (bf16); w_proj -> transposed DRAM scratch (f32r).
  Phase B: per superblock of 4 heads: V = x @ w_v^T + b_v (natural [t, d]
           layout, with an appended ones column per head for the softmax
           denominator); per head: Q^T/K^T = (w x^T) + b (head-aligned [d, t]
           layout, Q pre-scaled by 1/sqrt(hs) via prescaled weights); causal
           attention in S^T layout:
             S^T[j, i] = K^T.T-free matmul; P = exp(S^T) on ACT;
             diagonal-block mask multiply on DVE;
             O^T[d, i] (+ denominator row l) accumulated in PSUM over j-tiles
             via lhsT=[V|1]; normalize by 1/l via the single-pass DVE
             reciprocal_approx_fast (~5x faster than the exact iterative
             divide, keeps the ACT queue free for exp), spill O^T to DRAM
             per i-half so phase C can start early.
  Phase C: out = oT.T @ w_proj^T + b_proj with K=128 contraction tiles,
           written [t, c]; shares the phase-B scope so its matmuls overlap
           the tail of the attention.
"""

import math
import os
import sys
from contextlib import ExitStack

for _p in ("/opt/trn_rl_repo", "/root/.axon_site/_ro/trn_rl_repo"):
    if os.path.isdir(_p) and _p not in sys.path:
        sys.path.append(_p)

import numpy as np
import ml_dtypes

import concourse.bass as bass  # noqa: F401  (import keeps bass registered)
from concourse import bacc
import concourse.mybir as mybir
import concourse.tile as tile
from concourse.bass_utils import run_bass_kernel_spmd

F32 = mybir.dt.float32
F32R = mybir.dt.float32r
BF16 = mybir.dt.bfloat16
EXP = mybir.ActivationFunctionType.Exp
LN = mybir.ActivationFunctionType.Ln
ADD = mybir.AluOpType.add
MULT = mybir.AluOpType.mult

B, T, C, H, HS = 8, 2048, 768, 8, 96
KT = C // 128        # 6 contraction tiles of 128
TT = T // 128        # 16 t-tiles of 128
NCORES = 8


def _chunks(lo, hi, align=512):
    """Split [lo, hi) at multiples of `align`."""
    out = []
    a = lo
    while a < hi:
        b = min(hi, (a // align + 1) * align)
        out.append((a, b))
        a = b
    return out


def build_nc():
    nc = bacc.Bacc()
    x_b = nc.dram_tensor("x_b", [T, C], BF16, kind="ExternalInput")
    wat = nc.dram_tensor("wat", [3 * C, C], BF16, kind="ExternalInput")
    wp = nc.dram_tensor("wp", [C, C], F32R, kind="ExternalInput")
    ident = nc.dram_tensor("ident", [128, 128], BF16, kind="ExternalInput")
    identf = nc.dram_tensor("identf", [128, 128], F32R, kind="ExternalInput")
    mk = nc.dram_tensor("mk", [128, 128], F32R, kind="ExternalInput")
    bsel = nc.dram_tensor("bsel", [128, HS], F32R, kind="ExternalInput")
    bqk = nc.dram_tensor("bqk", [HS, 16], F32, kind="ExternalInput")
    bv = nc.dram_tensor("bv", [128, C], F32, kind="ExternalInput")
    bo = nc.dram_tensor("bo", [128, C], F32, kind="ExternalInput")
    out = nc.dram_tensor("out", [T, C], F32, kind="ExternalOutput")

    with tile.TileContext(nc) as tc, ExitStack() as ctx:

        consts = ctx.enter_context(tc.tile_pool(name="consts", bufs=1))
        id_sb = consts.tile([128, 128], BF16, tag="id")
        idf_sb = consts.tile([128, 128], F32R, tag="idf")
        mk_sb = consts.tile([128, 128], F32R, tag="mk")
        bs_sb = consts.tile([128, HS], F32R, tag="bs")
        bqk_sb = consts.tile([HS, 16], F32, tag="bqk")
        bv_sb = consts.tile([128, C], F32, tag="bv")
        bo_sb = consts.tile([128, C], F32, tag="bo")
        one_f32 = consts.tile([128, 1], F32, tag="one")
        nc.vector.memset(one_f32[:], 1.0)
        # ident first: the very first transposes need it; other consts follow
        # the big phase-A loads so they don't delay the critical path
        nc.sync.dma_start(id_sb[:], ident[:, :])

        xTp = ctx.enter_context(tc.tile_pool(name="xT", bufs=1))
        xT = xTp.tile([128, KT, T], BF16, tag="xT")
        waTp = ctx.enter_context(tc.tile_pool(name="waT", bufs=1))
        waT = waTp.tile([128, KT, 3 * C], BF16, tag="waT")   # w_attn^T resident
        # w_proj^T and the attention output O^T stay SBUF-resident end to end:
        # the transpose evictions write wpT_sb directly, and per-head O^T
        # pieces are partition-shifted into the K=128-packed oS stripes that
        # phase C consumes — no DRAM round trip.
        wpTp = ctx.enter_context(tc.tile_pool(name="wpTsb", bufs=1))
        wpT_sb = wpTp.tile([128, KT, C], BF16, tag="wpTsb")
        oSp = ctx.enter_context(tc.tile_pool(name="oS", bufs=1))
        oS = oSp.tile([128, KT, T], BF16, tag="oS")

        # ---------------- Phase A: transposes ----------------
        # Sources are DMAd in stripes sized so the PE transposes start early;
        # transposed-row accumulation happens in PSUM, one eviction copy per
        # 128-row stripe of the transposed tensor.
        with tc.tile_pool(name="pa_in", bufs=3) as pin, \
             tc.tile_pool(name="pa_ps", bufs=4, space="PSUM") as pps, \
             tc.tile_pool(name="pa_pw", bufs=2, space="PSUM") as ppw:
            # w_attn first (its transpose feeds the projections): the V-head
            # stripes, then interleaved q/k stripes so early heads unblock
            # first; x after; w_proj last (only needed in phase C).
            wat_r = wat.rearrange("(a p) c -> p a c", p=128)
            x_r = x_b.rearrange("(a p) c -> p a c", p=128)

            def tr_group(src, sls, kc, dst, psp, idt, dt):
                """Transpose len(sls) consecutive 128-blocks of `src` stripes
                into one wide PSUM tile, then evict with a single copy."""
                n = len(sls)
                psb = psp.tile([128, n * 128], dt, tag="tps")
                bank = 1024 if dt == BF16 else 512
                for i, sl in enumerate(sls):
                    first = (i == 0) or \
                        (i * 128) // bank != ((i * 128 - 1) // bank)
                    last = (i == n - 1) or \
                        ((i + 1) * 128 - 1) // bank != (((i + 2) * 128 - 1) // bank)
                    nc.tensor.matmul(psb[:, i * 128:(i + 1) * 128],
                                     src[:, sl, kc * 128:(kc + 1) * 128],
                                     idt[:], is_transpose=True,
                                     start=first, stop=last)
                nc.any.tensor_copy(dst, psb[:])

            def w_stripes(tile_, rt_groups):
                for kc in range(KT):
                    for (sl0, rt0, n) in rt_groups:
                        tr_group(tile_, range(sl0, sl0 + n), kc,
                                 waT[:, kc, rt0 * 128:(rt0 + n) * 128],
                                 pps, id_sb, BF16)

            vch = pin.tile([128, 6, C], BF16, tag="ain", name="vch")
            nc.sync.dma_start(vch[:, 0:3, :], wat_r[:, 12:15, :])
            nc.sync.dma_start(vch[:, 3:6, :], wat_r[:, 15:18, :])
            nc.sync.dma_start(mk_sb[:], mk[:, :])
            nc.sync.dma_start(bs_sb[:], bsel[:, :])
            nc.sync.dma_start(bqk_sb[:], bqk[:, :])
            nc.sync.dma_start(bv_sb[:], bv[:, :])
            nc.sync.dma_start(bo_sb[:], bo[:, :])
            nc.sync.dma_start(idf_sb[:], identf[:, :])
            w_stripes(vch, [(0, 12, 3), (3, 15, 3)])

            def x_chunk(ch):
                xch = pin.tile([128, 6, C], BF16, tag="ain", name=f"xch{ch}")
                nch = 6 if ch < 2 else 4
                nh = nch // 2
                nc.scalar.dma_start(xch[:, 0:nh, :],
                                    x_r[:, ch * 6:ch * 6 + nh, :])
                nc.scalar.dma_start(xch[:, nh:nch, :],
                                    x_r[:, ch * 6 + nh:ch * 6 + nch, :])
                for kc in range(KT):
                    for (s0, s1) in ((0, nh), (nh, nch)):
                        tr_group(xch, range(s0, s1), kc,
                                 xT[:, kc, (ch * 6 + s0) * 128:
                                    (ch * 6 + s1) * 128],
                                 pps, id_sb, BF16)

            x_chunk(0)
            # early heads' q/k stripes land between the x chunks so head 0's
            # projection can start as soon as the first x chunk is transposed
            qk0 = pin.tile([128, 6, C], BF16, tag="ain", name="qk0")
            nc.sync.dma_start(qk0[:, 0:3, :], wat_r[:, 0:3, :])
            nc.sync.dma_start(qk0[:, 3:6, :], wat_r[:, 6:9, :])
            w_stripes(qk0, [(0, 0, 3), (3, 6, 3)])
            x_chunk(1)
            qk1 = pin.tile([128, 6, C], BF16, tag="ain", name="qk1")
            nc.sync.dma_start(qk1[:, 0:3, :], wat_r[:, 3:6, :])
            nc.sync.dma_start(qk1[:, 3:6, :], wat_r[:, 9:12, :])
            w_stripes(qk1, [(0, 3, 3), (3, 9, 3)])
            x_chunk(2)
            wpin = pin.tile([128, 6, C], F32R, tag="win", name="wpin", bufs=1)
            nc.gpsimd.dma_start(wpin[:], wp.rearrange("(a p) c -> p a c", p=128))
            for kc in range(KT):
                tr_group(wpin, range(C // 128), kc, wpT_sb[:, kc, :],
                         ppw, idf_sb, F32R)

        # ------- Phase B: projections + attention;  Phase C: out proj -------
        # One scope so phase C's matmuls overlap the tail of the attention.
        with tc.tile_pool(name="vsb", bufs=2) as vsbp, \
             tc.tile_pool(name="qk", bufs=4) as qkp, \
             tc.tile_pool(name="pt", bufs=2) as ptp, \
             tc.tile_pool(name="ep", bufs=2) as epp, \
             tc.tile_pool(name="pcp", bufs=2) as pcp, \
             tc.tile_pool(name="bps", bufs=2, space="PSUM") as bps, \
             tc.tile_pool(name="pj", bufs=2, space="PSUM") as pjps, \
             tc.tile_pool(name="ops", bufs=1, space="PSUM") as opsp:
            for pr in range(4):
                # V projection for this pair of heads, natural [t, d] layout
                # with an appended ones column per head (softmax denominator).
                # The matmul N is padded to 256 (f32r needs free >= 256 for
                # full rate); the pad may read into neighbouring v columns.
                start_off = min(2 * HS * pr, C - 256)
                off = 2 * HS * pr - start_off
                V = vsbp.tile([128, TT, 2, HS + 1], BF16, tag="V")
                nc.vector.tensor_copy(
                    V.rearrange("p a b c -> p (a b c)"),
                    one_f32[:].to_broadcast([128, TT * 2 * (HS + 1)]))
                for tt in range(TT):
                    vps = pjps.tile([128, 512], F32, tag="pj")
                    for kc in range(KT):
                        nc.tensor.matmul(vps[:, 0:256],
                                         xT[:, kc, tt * 128:(tt + 1) * 128],
                                         waT[:, kc, 2 * C + start_off:
                                             2 * C + start_off + 256],
                                         start=(kc == 0), stop=(kc == KT - 1))
                    nc.vector.tensor_tensor(
                        V[:, tt, :, 0:HS],
                        vps[:, off:off + 2 * HS]
                            .rearrange("p (h d) -> p h d", d=HS),
                        bv_sb[:, 2 * HS * pr:2 * HS * (pr + 1)]
                            .rearrange("p (h d) -> p h d", d=HS),
                        ADD)

                for hh in range(2):
                    h = 2 * pr + hh
                    # Q^T/K^T projection for head h ([d, t] layout); per-head
                    # granularity lets the next head's projection overlap the
                    # current head's (ACT-bound) attention inner loop.
                    qkh = [qkp.tile([128, T], F32R, tag="qk", name=f"qk{i}")
                           for i in range(2)]
                    for tc4 in range(4):
                        for mc in range(2):          # 0 = q, 1 = k
                            wc = h * HS + (0 if mc == 0 else C)
                            pj = pjps.tile([128, 512], F32, tag="pj")
                            for kc in range(KT):
                                nc.tensor.matmul(
                                    pj[0:HS, 0:512],
                                    waT[:, kc, wc:wc + HS],
                                    xT[:, kc, tc4 * 512:(tc4 + 1) * 512],
                                    start=(kc == 0), stop=(kc == KT - 1))
                            m_col = h + (0 if mc == 0 else 8)
                            nc.vector.tensor_tensor(
                                qkh[mc][0:HS, tc4 * 512:(tc4 + 1) * 512],
                                pj[0:HS, 0:512],
                                bqk_sb[:, m_col:m_col + 1].to_broadcast([HS, 512]),
                                ADD)

                    qT, kT = qkh[0], qkh[1]
                    Oe = epp.tile([HS, T], BF16, tag="Oe", bufs=1)
                    for ihalf in range(2):
                        ibase = 1024 * ihalf
                        iend = ibase + 1024
                        njt = 8 * (ihalf + 1)
                        O_ps = opsp.tile([128, 1024], F32, tag="O")
                        for jt in range(njt):
                            j0 = 128 * jt
                            i0 = max(j0, ibase)
                            ilen = iend - i0
                            S = bps.tile([128, 1024], F32, tag="ps")
                            for (ra, rb) in _chunks(0, ilen):
                                nc.tensor.matmul(S[:, ra:rb],
                                                 kT[0:HS, j0:j0 + 128],
                                                 qT[0:HS, i0 + ra:i0 + rb],
                                                 start=True, stop=True)
                            P = ptp.tile([128, 1024], BF16, tag="P")
                            nc.scalar.activation(P[:, 0:ilen], S[:, 0:ilen],
                                                 EXP)
                            if j0 >= ibase:
                                nc.gpsimd.tensor_tensor(P[:, 0:128],
                                                        P[:, 0:128],
                                                        mk_sb[:], MULT)
                            for (a, b) in _chunks(i0, iend):
                                ci = a // 512
                                last_jt = min(4 * ci + 3, njt - 1)
                                nc.tensor.matmul(
                                    O_ps[0:HS + 1, a - ibase:b - ibase],
                                    V[:, jt, hh, :],
                                    P[:, a - i0:b - i0],
                                    start=(jt == 0), stop=(jt == last_jt))
                        # epilogue: normalize by the denominator row l
                        # (broadcast by selector matmul, then a single-pass
                        # approximate reciprocal on DVE — ~5x faster than the
                        # exact iterative divide, ~51 ULP).
                        lt = epp.tile([HS + 1, 1024], F32R, tag="lt",
                                      bufs=1)
                        nc.vector.tensor_copy(lt[:], O_ps[0:HS + 1, :])
                        Lp = bps.tile([128, 1024], F32, tag="ps")
                        for (ra, rb) in ((0, 512), (512, 1024)):
                            nc.tensor.matmul(Lp[0:HS, ra:rb],
                                             bs_sb[0:HS + 1, :],
                                             lt[:, ra:rb],
                                             start=True, stop=True)
                        R = epp.tile([HS, 1024], F32, tag="R", bufs=1)
                        nc.vector.reciprocal_approx_fast(R[:], Lp[0:HS, :])
                        nc.gpsimd.tensor_tensor(Oe[:, ibase:iend],
                                                lt[0:HS, :], R[:], MULT)
                        # shift this i-half into the K=128-packed stripes
                        # (SBUF->SBUF DMA does the partition remap)
                        r0 = h * HS
                        k0, off = r0 // 128, r0 % 128
                        n0 = min(HS, 128 - off)
                        nc.gpsimd.dma_start(
                            oS[off:off + n0, k0, ibase:iend],
                            Oe[0:n0, ibase:iend])
                        if n0 < HS:
                            nc.gpsimd.dma_start(
                                oS[0:HS - n0, k0 + 1, ibase:iend],
                                Oe[n0:HS, ibase:iend])

            # ---------------- Phase C: output projection ----------------
            # O^T streamed back from DRAM per 2-t-tile block (pipelined),
            # K=128 contraction tiles (128-row stripes of the concatenated
            # head dim), PSUM shared with the projection pool.
            out_r = out.rearrange("(g a p) c -> p g a c", a=2, p=128)
            # tg 0..3 (i < 1024) are fully unblocked once every head's first
            # i-half is done; run them whole.
            for tg in range(TT // 4):
                for ta in range(2):
                    t0 = tg * 256 + ta * 128
                    o_sb = pcp.tile([128, C], F32, tag="osb")
                    for (a, b) in ((0, 512), (512, C)):
                        cps = pjps.tile([128, 512], F32, tag="pj")
                        for kc in range(KT):
                            nc.tensor.matmul(cps[:, 0:b - a],
                                             oS[:, kc, t0:t0 + 128],
                                             wpT_sb[:, kc, a:b],
                                             start=(kc == 0), stop=(kc == KT - 1))
                        nc.vector.tensor_tensor(o_sb[:, a:b],
                                                cps[:, 0:b - a],
                                                bo_sb[:, a:b], ADD)
                    nc.gpsimd.dma_start(out_r[:, tg, ta], o_sb[:])
            # tg 4..7 (i >= 1024) would otherwise serialize behind the last
            # head's final i-half: contract kc 0..4 (+bias) into SBUF partials
            # while that head still runs, leaving only the kc=5 pass + add on
            # the tail.  All stage-1 allocations precede stage-2 so the psum
            # pool rotation never parks an early tile behind a late one.
            parts = {}
            for tg in range(TT // 4, TT // 2):
                for ta in range(2):
                    t0 = tg * 256 + ta * 128
                    part = pcp.tile([128, C], F32, tag="part", bufs=8)
                    parts[(tg, ta)] = part
                    for (a, b) in ((0, 512), (512, C)):
                        cps = pjps.tile([128, 512], F32, tag="pj")
                        for kc in range(KT - 1):
                            nc.tensor.matmul(cps[:, 0:b - a],
                                             oS[:, kc, t0:t0 + 128],
                                             wpT_sb[:, kc, a:b],
                                             start=(kc == 0), stop=(kc == KT - 2))
                        nc.vector.tensor_tensor(part[:, a:b],
                                                cps[:, 0:b - a],
                                                bo_sb[:, a:b], ADD)
            for tg in range(TT // 4, TT // 2):
                for ta in range(2):
                    t0 = tg * 256 + ta * 128
                    part = parts[(tg, ta)]
                    o_sb = pcp.tile([128, C], F32, tag="osb")
                    for (a, b) in ((0, 512), (512, C)):
                        cps = pjps.tile([128, 512], F32, tag="pj")
                        nc.tensor.matmul(cps[:, 0:b - a],
                                         oS[:, KT - 1, t0:t0 + 128],
                                         wpT_sb[:, KT - 1, a:b],
                                         start=True, stop=True)
                        nc.vector.tensor_tensor(o_sb[:, a:b],
                                                cps[:, 0:b - a],
                                                part[:, a:b], ADD)
                    nc.gpsimd.dma_start(out_r[:, tg, ta], o_sb[:])

    nc.finalize()
    return nc


_NC_CACHE = {}


def _get_nc():
    if "nc" not in _NC_CACHE:
        _NC_CACHE["nc"] = build_nc()
    return _NC_CACHE["nc"]


def _make_consts(b_attn, b_proj):
    s = 1.0 / math.sqrt(HS)
    bqk = np.empty((HS, 16), dtype=np.float32)
    for m in range(8):
        bqk[:, m] = b_attn[m * HS:(m + 1) * HS] * s
    for m in range(8):
        bqk[:, 8 + m] = b_attn[C + m * HS:C + (m + 1) * HS]
    bv = np.ascontiguousarray(
        np.broadcast_to(b_attn[2 * C:3 * C], (128, C)).astype(np.float32))
    bo = np.ascontiguousarray(
        np.broadcast_to(b_proj, (128, C)).astype(np.float32))
    ident = np.eye(128, dtype=np.float32)
    mk = np.triu(np.ones((128, 128), dtype=np.float32))
    bsel = np.zeros((128, HS), dtype=np.float32)
    bsel[HS, :] = 1.0
    return bqk, bv, bo, ident, mk, bsel


def kernel(x, w_attn, b_attn, w_proj, b_proj, _want_results=False, **run_kwargs):
    x = np.asarray(x, dtype=np.float32)
    w_attn = np.asarray(w_attn, dtype=np.float32)
    b_attn = np.asarray(b_attn, dtype=np.float32)
    w_proj = np.asarray(w_proj, dtype=np.float32)
    b_proj = np.asarray(b_proj, dtype=np.float32)

    s = 1.0 / math.sqrt(HS)
    wat = w_attn.copy()
    wat[0:C, :] *= s            # fold the 1/sqrt(hs) logit scale into Q
    wat_bf = wat.astype(ml_dtypes.bfloat16)
    x_bf = x.astype(ml_dtypes.bfloat16)
    bqk, bv, bo, ident, mk, bsel = _make_consts(b_attn, b_proj)

    nc = _get_nc()
    common = dict(wat=wat_bf, wp=w_proj,
                  ident=ident.astype(ml_dtypes.bfloat16), identf=ident,
                  mk=mk, bsel=bsel, bqk=bqk, bv=bv, bo=bo)
    in_maps = [dict(x_b=np.ascontiguousarray(x_bf[c]), **common)
               for c in range(NCORES)]
    res = run_bass_kernel_spmd(nc, in_maps, core_ids=list(range(NCORES)),
                               **run_kwargs)
    out = np.stack([res.results[c]["out"] for c in range(NCORES)], axis=0)
    if _want_results:
        return out, res
    return out


if __name__ == "__main__":
    rng = np.random.default_rng(0)
    x = rng.standard_normal((B, T, C), dtype=np.float32)
    w_attn = rng.standard_normal((3 * C, C), dtype=np.float32) / math.sqrt(C)
    b_attn = rng.standard_normal(3 * C).astype(np.float32) * 0.02
    w_proj = rng.standard_normal((C, C), dtype=np.float32) / math.sqrt(C)
    b_proj = rng.standard_normal(C).astype(np.float32) * 0.02
    o = kernel(x, w_attn, b_attn, w_proj, b_proj)
    print("out", o.shape, o.dtype, float(np.abs(o).mean()))


# revision 39
# speedup vs baseline: 1.2655x; 1.0791x over previous
"""Causal multi-head attention block (B=8, T=2048, C=768, H=8) on 8 trn2 cores.

Sharding: data-parallel over batch — one batch element per NeuronCore, weights
replicated, no collectives.

Per-core algorithm:
  Phase A: PE-transpose x_b -> x^T [c, t] in SBUF (bf16); w_attn -> transposed
           SBUF-resident waT (bf16); w_proj -> transposed DRAM scratch (f32r).
  Phase B: per superblock of 4 heads: V = x @ w_v^T + b_v (natural [t, d]
           layout, with an appended ones column per head for the softmax
           denominator); per head: Q^T/K^T = (w x^T) + b (head-aligned [d, t]
           layout, Q pre-scaled by 1/sqrt(hs) via prescaled weights); causal
           attention in S^T layout:
             S^T[j, i] = K^T.T-free matmul; P = exp(S^T) on ACT;
             diagonal-block mask multiply on DVE;
             O^T[d, i] (+ denominator row l) accumulated in PSUM over j-tiles
             via lhsT=[V|1]; normalize by 1/l via the single-pass DVE
             reciprocal_approx_fast (~5x faster than the exact iterative
             divide, keeps the ACT queue free for exp), spill O^T to DRAM
             per i-half so phase C can start early.
  Phase C: out = oT.T @ w_proj^T + b_proj with K=128 contraction tiles,
           written [t, c]; shares the phase-B scope so its matmuls overlap
           the tail of the attention.
"""

import math
import os
import sys
from contextlib import ExitStack

for _p in ("/opt/trn_rl_repo", "/root/.axon_site/_ro/trn_rl_repo"):
    if os.path.isdir(_p) and _p not in sys.path:
        sys.path.append(_p)

import numpy as np
import ml_dtypes

import concourse.bass as bass  # noqa: F401  (import keeps bass registered)
from concourse import bacc
import concourse.mybir as mybir
import concourse.tile as tile
from concourse.bass_utils import run_bass_kernel_spmd

F32 = mybir.dt.float32
F32R = mybir.dt.float32r
BF16 = mybir.dt.bfloat16
EXP = mybir.ActivationFunctionType.Exp
LN = mybir.ActivationFunctionType.Ln
ADD = mybir.AluOpType.add
MULT = mybir.AluOpType.mult

B, T, C, H, HS = 8, 2048, 768, 8, 96
KT = C // 128        # 6 contraction tiles of 128
TT = T // 128        # 16 t-tiles of 128
NCORES = 8


def _chunks(lo, hi, align=512):
    """Split [lo, hi) at multiples of `align`."""
    out = []
    a = lo
    while a < hi:
        b = min(hi, (a // align + 1) * align)
        out.append((a, b))
        a = b
    return out


def build_nc():
    nc = bacc.Bacc()
    x_b = nc.dram_tensor("x_b", [T, C], BF16, kind="ExternalInput")
    wat = nc.dram_tensor("wat", [3 * C, C], BF16, kind="ExternalInput")
    wp = nc.dram_tensor("wp", [C, C], F32R, kind="ExternalInput")
    ident = nc.dram_tensor("ident", [128, 128], BF16, kind="ExternalInput")
    identf = nc.dram_tensor("identf", [128, 128], F32R, kind="ExternalInput")
    mk = nc.dram_tensor("mk", [128, 128], F32R, kind="ExternalInput")
    bsel = nc.dram_tensor("bsel", [128, HS], F32R, kind="ExternalInput")
    bqk = nc.dram_tensor("bqk", [HS, 16], F32, kind="ExternalInput")
    bv = nc.dram_tensor("bv", [128, C], F32, kind="ExternalInput")
    bo = nc.dram_tensor("bo", [128, C], F32, kind="ExternalInput")
    out = nc.dram_tensor("out", [T, C], F32, kind="ExternalOutput")

    with tile.TileContext(nc) as tc, ExitStack() as ctx:

        consts = ctx.enter_context(tc.tile_pool(name="consts", bufs=1))
        id_sb = consts.tile([128, 128], BF16, tag="id")
        idf_sb = consts.tile([128, 128], F32R, tag="idf")
        mk_sb = consts.tile([128, 128], F32R, tag="mk")
        bs_sb = consts.tile([128, HS], F32R, tag="bs")
        bqk_sb = consts.tile([HS, 16], F32, tag="bqk")
        bv_sb = consts.tile([128, C], F32, tag="bv")
        bo_sb = consts.tile([128, C], F32, tag="bo")
        one_f32 = consts.tile([128, 1], F32, tag="one")
        nc.vector.memset(one_f32[:], 1.0)
        # ident first: the very first transposes need it; other consts follow
        # the big phase-A loads so they don't delay the critical path
        nc.sync.dma_start(id_sb[:], ident[:, :])

        xTp = ctx.enter_context(tc.tile_pool(name="xT", bufs=1))
        xT = xTp.tile([128, KT, T], BF16, tag="xT")
        waTp = ctx.enter_context(tc.tile_pool(name="waT", bufs=1))
        waT = waTp.tile([128, KT, 3 * C], BF16, tag="waT")   # w_attn^T resident
        # w_proj^T and the attention output O^T stay SBUF-resident end to end:
        # the transpose evictions write wpT_sb directly, and per-head O^T
        # pieces are partition-shifted into the K=128-packed oS stripes that
        # phase C consumes — no DRAM round trip.
        wpTp = ctx.enter_context(tc.tile_pool(name="wpTsb", bufs=1))
        wpT_sb = wpTp.tile([128, KT, C], BF16, tag="wpTsb")
        oSp = ctx.enter_context(tc.tile_pool(name="oS", bufs=1))
        oS = oSp.tile([128, KT, T], BF16, tag="oS")

        # ---------------- Phase A: transposes ----------------
        # Sources are DMAd in stripes sized so the PE transposes start early;
        # transposed-row accumulation happens in PSUM, one eviction copy per
        # 128-row stripe of the transposed tensor.
        with tc.tile_pool(name="pa_in", bufs=3) as pin, \
             tc.tile_pool(name="pa_ps", bufs=4, space="PSUM") as pps, \
             tc.tile_pool(name="pa_pw", bufs=2, space="PSUM") as ppw:
            # w_attn first (its transpose feeds the projections): the V-head
            # stripes, then interleaved q/k stripes so early heads unblock
            # first; x after; w_proj last (only needed in phase C).
            wat_r = wat.rearrange("(a p) c -> p a c", p=128)
            x_r = x_b.rearrange("(a p) c -> p a c", p=128)

            def tr_group(src, sls, kc, dst, psp, idt, dt):
                """Transpose len(sls) consecutive 128-blocks of `src` stripes
                into one wide PSUM tile, then evict with a single copy."""
                n = len(sls)
                psb = psp.tile([128, n * 128], dt, tag="tps")
                bank = 1024 if dt == BF16 else 512
                for i, sl in enumerate(sls):
                    first = (i == 0) or \
                        (i * 128) // bank != ((i * 128 - 1) // bank)
                    last = (i == n - 1) or \
                        ((i + 1) * 128 - 1) // bank != (((i + 2) * 128 - 1) // bank)
                    nc.tensor.matmul(psb[:, i * 128:(i + 1) * 128],
                                     src[:, sl, kc * 128:(kc + 1) * 128],
                                     idt[:], is_transpose=True,
                                     start=first, stop=last)
                nc.any.tensor_copy(dst, psb[:])

            def w_stripes(tile_, rt_groups):
                for kc in range(KT):
                    for (sl0, rt0, n) in rt_groups:
                        tr_group(tile_, range(sl0, sl0 + n), kc,
                                 waT[:, kc, rt0 * 128:(rt0 + n) * 128],
                                 pps, id_sb, BF16)

            vch = pin.tile([128, 6, C], BF16, tag="ain", name="vch")
            nc.sync.dma_start(vch[:, 0:3, :], wat_r[:, 12:15, :])
            nc.sync.dma_start(vch[:, 3:6, :], wat_r[:, 15:18, :])
            nc.sync.dma_start(mk_sb[:], mk[:, :])
            nc.sync.dma_start(bs_sb[:], bsel[:, :])
            nc.sync.dma_start(bqk_sb[:], bqk[:, :])
            nc.sync.dma_start(bv_sb[:], bv[:, :])
            nc.sync.dma_start(bo_sb[:], bo[:, :])
            nc.sync.dma_start(idf_sb[:], identf[:, :])
            w_stripes(vch, [(0, 12, 6)])

            def x_chunk(ch):
                xch = pin.tile([128, 6, C], BF16, tag="ain", name=f"xch{ch}")
                nch = 6 if ch < 2 else 4
                nh = nch // 2
                nc.scalar.dma_start(xch[:, 0:nh, :],
                                    x_r[:, ch * 6:ch * 6 + nh, :])
                nc.scalar.dma_start(xch[:, nh:nch, :],
                                    x_r[:, ch * 6 + nh:ch * 6 + nch, :])
                for kc in range(KT):
                    tr_group(xch, range(nch), kc,
                             xT[:, kc, ch * 6 * 128:(ch * 6 + nch) * 128],
                             pps, id_sb, BF16)

            x_chunk(0)
            # early heads' q/k stripes land between the x chunks so head 0's
            # projection can start as soon as the first x chunk is transposed
            qk0 = pin.tile([128, 6, C], BF16, tag="ain", name="qk0")
            nc.sync.dma_start(qk0[:, 0:3, :], wat_r[:, 0:3, :])
            nc.sync.dma_start(qk0[:, 3:6, :], wat_r[:, 6:9, :])
            w_stripes(qk0, [(0, 0, 3), (3, 6, 3)])
            x_chunk(1)
            qk1 = pin.tile([128, 6, C], BF16, tag="ain", name="qk1")
            nc.sync.dma_start(qk1[:, 0:3, :], wat_r[:, 3:6, :])
            nc.sync.dma_start(qk1[:, 3:6, :], wat_r[:, 9:12, :])
            w_stripes(qk1, [(0, 3, 3), (3, 9, 3)])
            x_chunk(2)
            wpin = pin.tile([128, 6, C], F32R, tag="win", name="wpin", bufs=1)
            nc.gpsimd.dma_start(wpin[:], wp.rearrange("(a p) c -> p a c", p=128))
            for kc in range(KT):
                tr_group(wpin, range(C // 128), kc, wpT_sb[:, kc, :],
                         ppw, idf_sb, F32R)

        # ------- Phase B: projections + attention;  Phase C: out proj -------
        # One scope so phase C's matmuls overlap the tail of the attention.
        with tc.tile_pool(name="vsb", bufs=2) as vsbp, \
             tc.tile_pool(name="qk", bufs=4) as qkp, \
             tc.tile_pool(name="pt", bufs=2) as ptp, \
             tc.tile_pool(name="ep", bufs=2) as epp, \
             tc.tile_pool(name="pcp", bufs=2) as pcp, \
             tc.tile_pool(name="bps", bufs=2, space="PSUM") as bps, \
             tc.tile_pool(name="pj", bufs=2, space="PSUM") as pjps, \
             tc.tile_pool(name="ops", bufs=1, space="PSUM") as opsp:
            for pr in range(4):
                # V projection for this pair of heads, natural [t, d] layout
                # with an appended ones column per head (softmax denominator).
                # bf16 matmuls run at full rate at any free size, so N is the
                # exact 192 pair width — no padding.
                V = vsbp.tile([128, TT, 2, HS + 1], BF16, tag="V")
                nc.vector.tensor_copy(
                    V.rearrange("p a b c -> p (a b c)"),
                    one_f32[:].to_broadcast([128, TT * 2 * (HS + 1)]))
                for tt in range(TT):
                    vps = pjps.tile([128, 512], F32, tag="pj")
                    for kc in range(KT):
                        nc.tensor.matmul(vps[:, 0:2 * HS],
                                         xT[:, kc, tt * 128:(tt + 1) * 128],
                                         waT[:, kc, 2 * C + 2 * HS * pr:
                                             2 * C + 2 * HS * (pr + 1)],
                                         start=(kc == 0), stop=(kc == KT - 1))
                    nc.vector.tensor_tensor(
                        V[:, tt, :, 0:HS],
                        vps[:, 0:2 * HS]
                            .rearrange("p (h d) -> p h d", d=HS),
                        bv_sb[:, 2 * HS * pr:2 * HS * (pr + 1)]
                            .rearrange("p (h d) -> p h d", d=HS),
                        ADD)

                for hh in range(2):
                    h = 2 * pr + hh
                    # Q^T/K^T projection for head h ([d, t] layout); per-head
                    # granularity lets the next head's projection overlap the
                    # current head's (ACT-bound) attention inner loop.
                    qkh = [qkp.tile([128, T], BF16, tag="qk", name=f"qk{i}")
                           for i in range(2)]
                    for tc4 in range(4):
                        for mc in range(2):          # 0 = q, 1 = k
                            wc = h * HS + (0 if mc == 0 else C)
                            pj = pjps.tile([128, 512], F32, tag="pj")
                            for kc in range(KT):
                                nc.tensor.matmul(
                                    pj[0:HS, 0:512],
                                    waT[:, kc, wc:wc + HS],
                                    xT[:, kc, tc4 * 512:(tc4 + 1) * 512],
                                    start=(kc == 0), stop=(kc == KT - 1))
                            m_col = h + (0 if mc == 0 else 8)
                            nc.vector.tensor_tensor(
                                qkh[mc][0:HS, tc4 * 512:(tc4 + 1) * 512],
                                pj[0:HS, 0:512],
                                bqk_sb[:, m_col:m_col + 1].to_broadcast([HS, 512]),
                                ADD)

                    qT, kT = qkh[0], qkh[1]
                    Oe = epp.tile([HS, T], BF16, tag="Oe", bufs=1)
                    for ihalf in range(2):
                        ibase = 1024 * ihalf
                        iend = ibase + 1024
                        njt = 8 * (ihalf + 1)
                        O_ps = opsp.tile([128, 1024], F32, tag="O")
                        for jt in range(njt):
                            j0 = 128 * jt
                            i0 = max(j0, ibase)
                            ilen = iend - i0
                            S = bps.tile([128, 1024], F32, tag="ps")
                            for (ra, rb) in _chunks(0, ilen):
                                nc.tensor.matmul(S[:, ra:rb],
                                                 kT[0:HS, j0:j0 + 128],
                                                 qT[0:HS, i0 + ra:i0 + rb],
                                                 start=True, stop=True)
                            P = ptp.tile([128, 1024], BF16, tag="P")
                            nc.scalar.activation(P[:, 0:ilen], S[:, 0:ilen],
                                                 EXP)
                            if j0 >= ibase:
                                nc.gpsimd.tensor_tensor(P[:, 0:128],
                                                        P[:, 0:128],
                                                        mk_sb[:], MULT)
                            for (a, b) in _chunks(i0, iend):
                                ci = a // 512
                                last_jt = min(4 * ci + 3, njt - 1)
                                nc.tensor.matmul(
                                    O_ps[0:HS + 1, a - ibase:b - ibase],
                                    V[:, jt, hh, :],
                                    P[:, a - i0:b - i0],
                                    start=(jt == 0), stop=(jt == last_jt))
                        # epilogue: normalize by the denominator row l
                        # (broadcast by selector matmul, then a single-pass
                        # approximate reciprocal on DVE — ~5x faster than the
                        # exact iterative divide, ~51 ULP).
                        lt = epp.tile([HS + 1, 1024], F32R, tag="lt",
                                      bufs=1)
                        nc.vector.tensor_copy(lt[:], O_ps[0:HS + 1, :])
                        Lp = bps.tile([128, 1024], F32, tag="ps")
                        for (ra, rb) in ((0, 512), (512, 1024)):
                            nc.tensor.matmul(Lp[0:HS, ra:rb],
                                             bs_sb[0:HS + 1, :],
                                             lt[:, ra:rb],
                                             start=True, stop=True)
                        R = epp.tile([HS, 1024], F32, tag="R", bufs=1)
                        nc.vector.reciprocal_approx_fast(R[:], Lp[0:HS, :])
                        nc.gpsimd.tensor_tensor(Oe[:, ibase:iend],
                                                lt[0:HS, :], R[:], MULT)
                        # shift this i-half into the K=128-packed stripes
                        # (SBUF->SBUF DMA does the partition remap)
                        r0 = h * HS
                        k0, off = r0 // 128, r0 % 128
                        n0 = min(HS, 128 - off)
                        nc.gpsimd.dma_start(
                            oS[off:off + n0, k0, ibase:iend],
                            Oe[0:n0, ibase:iend])
                        if n0 < HS:
                            nc.gpsimd.dma_start(
                                oS[0:HS - n0, k0 + 1, ibase:iend],
                                Oe[n0:HS, ibase:iend])

            # ---------------- Phase C: output projection ----------------
            # O^T streamed back from DRAM per 2-t-tile block (pipelined),
            # K=128 contraction tiles (128-row stripes of the concatenated
            # head dim), PSUM shared with the projection pool.
            out_r = out.rearrange("(g a p) c -> p g a c", a=2, p=128)
            for tg in range(TT // 2):
                for ta in range(2):
                    t0 = tg * 256 + ta * 128
                    o_sb = pcp.tile([128, C], F32, tag="osb")
                    for (a, b) in ((0, 512), (512, C)):
                        cps = pjps.tile([128, 512], F32, tag="pj")
                        for kc in range(KT):
                            nc.tensor.matmul(cps[:, 0:b - a],
                                             oS[:, kc, t0:t0 + 128],
                                             wpT_sb[:, kc, a:b],
                                             start=(kc == 0), stop=(kc == KT - 1))
                        nc.vector.tensor_tensor(o_sb[:, a:b],
                                                cps[:, 0:b - a],
                                                bo_sb[:, a:b], ADD)
                    nc.gpsimd.dma_start(out_r[:, tg, ta], o_sb[:])

    nc.finalize()
    return nc


_NC_CACHE = {}


def _get_nc():
    if "nc" not in _NC_CACHE:
        _NC_CACHE["nc"] = build_nc()
    return _NC_CACHE["nc"]


def _make_consts(b_attn, b_proj):
    s = 1.0 / math.sqrt(HS)
    bqk = np.empty((HS, 16), dtype=np.float32)
    for m in range(8):
        bqk[:, m] = b_attn[m * HS:(m + 1) * HS] * s
    for m in range(8):
        bqk[:, 8 + m] = b_attn[C + m * HS:C + (m + 1) * HS]
    bv = np.ascontiguousarray(
        np.broadcast_to(b_attn[2 * C:3 * C], (128, C)).astype(np.float32))
    bo = np.ascontiguousarray(
        np.broadcast_to(b_proj, (128, C)).astype(np.float32))
    ident = np.eye(128, dtype=np.float32)
    mk = np.triu(np.ones((128, 128), dtype=np.float32))
    bsel = np.zeros((128, HS), dtype=np.float32)
    bsel[HS, :] = 1.0
    return bqk, bv, bo, ident, mk, bsel


def kernel(x, w_attn, b_attn, w_proj, b_proj, _want_results=False, **run_kwargs):
    x = np.asarray(x, dtype=np.float32)
    w_attn = np.asarray(w_attn, dtype=np.float32)
    b_attn = np.asarray(b_attn, dtype=np.float32)
    w_proj = np.asarray(w_proj, dtype=np.float32)
    b_proj = np.asarray(b_proj, dtype=np.float32)

    s = 1.0 / math.sqrt(HS)
    wat = w_attn.copy()
    wat[0:C, :] *= s            # fold the 1/sqrt(hs) logit scale into Q
    wat_bf = wat.astype(ml_dtypes.bfloat16)
    x_bf = x.astype(ml_dtypes.bfloat16)
    bqk, bv, bo, ident, mk, bsel = _make_consts(b_attn, b_proj)

    nc = _get_nc()
    common = dict(wat=wat_bf, wp=w_proj,
                  ident=ident.astype(ml_dtypes.bfloat16), identf=ident,
                  mk=mk, bsel=bsel, bqk=bqk, bv=bv, bo=bo)
    in_maps = [dict(x_b=np.ascontiguousarray(x_bf[c]), **common)
               for c in range(NCORES)]
    res = run_bass_kernel_spmd(nc, in_maps, core_ids=list(range(NCORES)),
                               **run_kwargs)
    out = np.stack([res.results[c]["out"] for c in range(NCORES)], axis=0)
    if _want_results:
        return out, res
    return out


if __name__ == "__main__":
    rng = np.random.default_rng(0)
    x = rng.standard_normal((B, T, C), dtype=np.float32)
    w_attn = rng.standard_normal((3 * C, C), dtype=np.float32) / math.sqrt(C)
    b_attn = rng.standard_normal(3 * C).astype(np.float32) * 0.02
    w_proj = rng.standard_normal((C, C), dtype=np.float32) / math.sqrt(C)
    b_proj = rng.standard_normal(C).astype(np.float32) * 0.02
    o = kernel(x, w_attn, b_attn, w_proj, b_proj)
    print("out", o.shape, o.dtype, float(np.abs(o).mean()))
